# revision 23
# baseline (speedup 1.0000x reference)
"""Trainium2 Bass kernel for GQA causal attention (B=2, S=2048, D=2048,
16 q-heads / 4 kv-heads, head_dim=128, interleaved RoPE).

Sharding: DP=2 over batch x TP=4 over head groups (8 cores).
Core c: batch b=c//4, rank r=c%4 -> q-heads [4r,4r+4), kv-head r.
Each core computes its heads' attention output (transposed layout [e,s]),
two column-strip AllToAlls reshard heads->sequence (overlapped with the
tail of attention), and each core runs the full output projection for its
512 strided sequence rows. Host-side work is layout only: slicing,
transposing, bf16 casting.

v2 changes vs baseline (trace-driven):
 - RoPE: partition-half swap via one PE permutation matmul instead of
   SBUF->SBUF DMAs; 3 full-height bf16 DVE ops (2x mode) instead of six
   half-height fp32 ops (1x).
 - softmax denominator: fp16 accumulate on wide group tiles (2x DVE),
   reciprocal via reciprocal_approx_fast (was 4us/call DVE reciprocal).
 - scores: 3-bank / 2-bank wide PSUM group tiles, ONE exp ACTIVATE per
   group (amortizes the 352-cycle ACT overhead).
 - exp ACT table preloaded at graph start.
 - a2a export/import as single rearranged-AP DMAs instead of 4-16 small
   DMAs (Sync engine issue cost).
"""

import math
import sys

sys.path.insert(0, "/opt/trn_rl_repo")

from contextlib import ExitStack

import ml_dtypes
import numpy as np

import concourse.bass as bass
import concourse.mybir as mybir
import concourse.tile as tile
from concourse import bacc
from concourse.bass_utils import run_bass_kernel_spmd
from concourse.masks import make_identity

BF16 = mybir.dt.bfloat16
F16 = mybir.dt.float16
F32 = mybir.dt.float32

N_HEADS = 16
N_KV_HEADS = 4
HD = 128
ROPE_THETA = 10000.0
TP = 4
N_CORES = 8


def build_graph(S=2048, D=2048, HQL=4, NS=512):
    """Per-core SPMD graph. HQL = local q heads; local kv heads = 1.

    Output ownership is strided by 128-col strips: core c owns sequence
    cols {c*128 + m*1024} of both batches; strip set m is exchanged by
    AllToAll #m as soon as the first half of attention chunks finish.
    """
    hd = HD
    ND = D // 128          # d-tiles (projection contraction tiles)
    NC = S // NS           # s-chunks
    NK = S // 128          # sk-tiles
    MQ = HQL * hd          # local q width
    DIAG = NS // 128       # sk-tiles per chunk needing a causal mask
    NB = N_CORES // TP     # batches
    OW = S // N_CORES      # out cols per core per batch
    NM = max(1, S // (N_CORES * 128))   # strips (AllToAll count)
    SW = OW // NM          # strip width (=128 at full size)
    NSW = NS // SW         # strips exported per pass
    scale = 1.0 / math.sqrt(hd)
    NH = TP * HQL          # global head count

    nc = bacc.Bacc("TRN2", target_bir_lowering=False, debug=False,
                   num_devices=N_CORES)

    xT_e = nc.dram_tensor("xT", [D, S], BF16, kind="ExternalInput").ap()
    wqT_e = nc.dram_tensor("wqT", [D, MQ], BF16, kind="ExternalInput").ap()
    wkT_e = nc.dram_tensor("wkT", [D, hd], BF16, kind="ExternalInput").ap()
    wvT_e = nc.dram_tensor("wvT", [D, hd], BF16, kind="ExternalInput").ap()
    woT_e = nc.dram_tensor("woT", [NH * hd, D], BF16,
                           kind="ExternalInput").ap()
    ccx_e = nc.dram_tensor("ccx", [128, S], BF16, kind="ExternalInput").ap()
    ssx_e = nc.dram_tensor("ssx", [128, S], BF16, kind="ExternalInput").ap()
    psw_e = nc.dram_tensor("pswap", [128, 128], BF16,
                           kind="ExternalInput").ap()
    mask_e = nc.dram_tensor("mask", [128, NS + 384], F16,
                            kind="ExternalInput").ap()
    out_e = nc.dram_tensor("out", [NB * OW, D], F32,
                           kind="ExternalOutput").ap()

    a2a_in = [nc.dram_tensor(f"a2a_in{m}", [N_CORES * MQ, SW], BF16)
              for m in range(NM)]
    a2a_out = [nc.dram_tensor(f"a2a_out{m}", [N_CORES * MQ, SW], BF16)
               for m in range(NM)]
    groups = [list(range(N_CORES))]

    with tile.TileContext(nc) as tc, ExitStack() as ctx:
        ep = ctx.enter_context
        const_pool = ep(tc.tile_pool(name="const", bufs=1))
        rt_pool = ep(tc.tile_pool(name="rt", bufs=HQL + 1))
        vst_pool = ep(tc.tile_pool(name="vst", bufs=1))
        ptw_pool = ep(tc.tile_pool(name="ptw", bufs=3))
        den_pool = ep(tc.tile_pool(name="den", bufs=3))
        rc_pool = ep(tc.tile_pool(name="rc", bufs=3))
        rbc_pool = ep(tc.tile_pool(name="rbc", bufs=3))
        attn_pool = ep(tc.tile_pool(name="attn", bufs=4))
        osb_pool = ep(tc.tile_pool(name="osb", bufs=3))
        # PSUM: 8 banks total, statically partitioned:
        #   scw3 1x3 banks + scw2 1x2 + atp 2x1 + dps 1x1 = 8
        scw3_pool = ep(tc.tile_pool(name="scw3", bufs=1, space="PSUM"))
        scw2_pool = ep(tc.tile_pool(name="scw2", bufs=1, space="PSUM"))
        atp_pool = ep(tc.tile_pool(name="atp", bufs=2, space="PSUM"))
        dps_pool = ep(tc.tile_pool(name="dps", bufs=1, space="PSUM"))

        # ---- constants ----
        ident = const_pool.tile([128, 128], BF16, tag="ident")
        make_identity(nc, ident[:])
        ones = const_pool.tile([128, 1], F16, tag="ones")
        nc.gpsimd.memset(ones[:], 1.0)
        warm = const_pool.tile([1, 8], F32, tag="warm")
        nc.gpsimd.memset(warm[:], 0.0)
        # preload the exp ACT table set before attention needs it
        nc.scalar.activation(warm[:], warm[:],
                             mybir.ActivationFunctionType.Exp)
        ccx = const_pool.tile([128, S], BF16, tag="ccx")
        ssx = const_pool.tile([128, S], BF16, tag="ssx")
        pswap = const_pool.tile([128, 128], BF16, tag="pswap")
        msk = const_pool.tile([128, NS + 384], F16, tag="msk")

        rts = []
        vst = vst_pool.tile([128, S], BF16, tag="vst")   # vT staging
        vnat = vst_pool.tile([128, S], BF16, tag="vnat")  # v [sk, e] blocks

        # ---- phase 1: projections + rope (xt pools close after) ----
        with tc.tile_pool(name="xt", bufs=ND) as xt_pool, \
             tc.tile_pool(name="wq", bufs=ND) as wq_pool, \
             tc.tile_pool(name="wkv", bufs=2 * ND) as wkv_pool, \
             tc.tile_pool(name="stg", bufs=2) as stage_pool, \
             tc.tile_pool(name="tmp", bufs=4) as tmp_pool:
            xts, wqs, wks, wvs = [], [], [], []
            # xt+wq interleaved first (q heads project first and are
            # DMA-gated); v/k weights after
            for d in range(ND):
                xt = xt_pool.tile([128, S], BF16, tag="xt",
                                  name=f"xt{d}")
                nc.sync.dma_start(xt[:], xT_e[d * 128:(d + 1) * 128, :])
                xts.append(xt)
                wq = wq_pool.tile([128, MQ], BF16, tag="wq", name=f"wq{d}")
                nc.sync.dma_start(wq[:], wqT_e[d * 128:(d + 1) * 128, :])
                wqs.append(wq)
                if d == 0:
                    # constants queue behind the first xt/wq pair so
                    # they don't delay the first projection matmul
                    nc.sync.dma_start(ccx[:], ccx_e[:])
                    nc.sync.dma_start(ssx[:], ssx_e[:])
                    nc.sync.dma_start(pswap[:], psw_e[:])
                    nc.sync.dma_start(msk[:], mask_e[:])
            for d in range(ND):
                wv = wkv_pool.tile([128, hd], BF16, tag="wkv",
                                   name=f"wv{d}")
                nc.sync.dma_start(wv[:], wvT_e[d * 128:(d + 1) * 128, :])
                wvs.append(wv)
            for d in range(ND):
                wk = wkv_pool.tile([128, hd], BF16, tag="wkv",
                                   name=f"wk{d}")
                nc.sync.dma_start(wk[:], wkT_e[d * 128:(d + 1) * 128, :])
                wks.append(wk)

            # Rope is software-pipelined one m-tile behind the
            # projection matmuls: the swap-MM of m-tile k is emitted
            # after the proj group of m-tile k+1 so it never stalls the
            # PE waiting for the ACT psum->stg copy.
            rope_pend = []   # stack of (stg, ps2 emitter, rt_tile, ssl)

            def rope_tail():
                if not rope_pend:
                    return
                stg, ps2, rt_tile, ssl = rope_pend.pop(0)
                t1 = tmp_pool.tile([128, NS], BF16, tag="tmp")
                nc.vector.tensor_mul(t1[:], stg[:], ccx[:, ssl])
                t2 = tmp_pool.tile([128, NS], BF16, tag="tmp")
                nc.vector.tensor_mul(t2[:], ps2[:], ssx[:, ssl])
                nc.vector.tensor_add(rt_tile[:, ssl], t1[:], t2[:])

            def rope_swap_mm():
                if not rope_pend:
                    return
                stg, _, rt_tile, ssl = rope_pend[0]
                ps2 = scw2_pool.tile([128, NS], F32, tag="scw2",
                                     name="ps_swap")
                nc.tensor.matmul(ps2[:], pswap[:], stg[:],
                                 start=True, stop=True)
                rope_pend[0] = (stg, ps2, rt_tile, ssl)

            def proj_mtile(lhs_tiles, mslice, is_v, rt_tile):
                for s in range(NC):
                    ssl = slice(s * NS, (s + 1) * NS)
                    ps = atp_pool.tile([128, NS], F32, tag="atp",
                                       name="psp")
                    for d in range(ND):
                        nc.tensor.matmul(
                            ps[:], lhs_tiles[d][:, mslice],
                            xts[d][:, ssl],
                            start=(d == 0), stop=(d == ND - 1))
                    # emit the PREVIOUS m-tile's swap-MM now (its stg
                    # copy has had a full m-tile of time to finish)
                    rope_swap_mm()
                    rope_tail()
                    if is_v:
                        nc.scalar.copy(vst[:, ssl], ps[:])
                    else:
                        stg = stage_pool.tile([128, NS], BF16, tag="stg")
                        nc.scalar.copy(stg[:], ps[:])
                        rope_pend.append((stg, None, rt_tile, ssl))

            # q heads first, then v, then k LAST: attention chunk j only
            # needs k-rope of chunk j, so scores start right after k's
            # first chunk instead of after the whole projection phase
            for h in range(HQL):
                rt = rt_pool.tile([128, S], BF16, tag="rt", name=f"rtq{h}")
                proj_mtile(wqs, slice(h * hd, (h + 1) * hd), False, rt)
                rts.append(rt)
            proj_mtile(wvs, slice(0, hd), True, None)
            # v transpose: vst [e, s] -> vnat [sk, e] blocks
            for st in range(NK):
                tpp = dps_pool.tile([128, 128], BF16, tag="dps",
                                    name="pst")
                if st == 0:
                    rope_swap_mm()
                    rope_tail()
                nc.tensor.transpose(
                    tpp[:], vst[:, st * 128:(st + 1) * 128], ident[:])
                nc.scalar.copy(vnat[:, st * 128:(st + 1) * 128], tpp[:])
            krt = rt_pool.tile([128, S], BF16, tag="rt", name="rtk")
            proj_mtile(wks, slice(0, hd), False, krt)
            rope_swap_mm()
            rope_tail()

        # ---- woT preload (streams during attention; reuses xt space) ----
        wo_pool = ep(tc.tile_pool(name="wo", bufs=NH))
        aot_pool = ep(tc.tile_pool(name="aot", bufs=2))
        wo_tiles = []
        for ht in range(NH):
            w = wo_pool.tile([128, D], BF16, tag="wo", name=f"wo{ht}")
            nc.sync.dma_start(w[:], woT_e[ht * 128:(ht + 1) * 128, :])
            wo_tiles.append(w)

        # ---- phase 2: attention ----
        # One head per pass; sk-tiles processed in wide groups that
        # alternate between a 3-bank and a 2-bank PSUM tile, with ONE
        # exp ACTIVATE per group. Score groups are emitted one step
        # ahead of the exp/mask/den/AV tail so the PE never waits on
        # ACT; a pass's finalize (den fold+reduce, recip, broadcast)
        # and export (normalize + DMA + a2a) are deferred into the next
        # pass's first two groups.
        class Pass:
            def __init__(self, j, h, g0):
                self.j = j
                self.h = h
                self.nsk = (j + 1) * DIAG
                # group sizes follow the GLOBAL 3/2 pool alternation so
                # consecutive passes never collide on the same PSUM slot
                self.gs = []
                rem = self.nsk
                g = g0
                while rem > 0:
                    take = min(3 if g % 2 == 0 else 2, rem)
                    self.gs.append((g, take))
                    rem -= take
                    g += 1
                self.g_end = g
                self.dw = 0      # initialized width (slots) of denw
                self.denw = None
                self.at_ps = None
                self.rbc = None

        def score_group(p, gidx, gi, si0, G):
            """score MMs + exp for one group; ACT starts ASAP."""
            pool = scw3_pool if gidx % 2 == 0 else scw2_pool
            tag = "scw3" if gidx % 2 == 0 else "scw2"
            scw = pool.tile([128, G * NS], F32, tag=tag,
                            name=f"sc_{p.j}_{p.h}_{gi}")
            for lg in range(G):
                si = si0 + lg
                nc.tensor.matmul(
                    scw[:, lg * NS:(lg + 1) * NS],
                    krt[:, si * 128:(si + 1) * 128],
                    rts[p.h][:, p.j * NS:(p.j + 1) * NS],
                    start=True, stop=True)
            ptw = ptw_pool.tile([128, G * NS], F16, tag="ptw")
            nc.scalar.activation(ptw[:], scw[:],
                                 mybir.ActivationFunctionType.Exp,
                                 scale=scale)
            return ptw

        def den_tail(p, gi, si0, G, ptw):
            """mask + den-accumulate for one group (DVE, lag 1)."""
            for lg in range(G):
                si = si0 + lg
                o = si * 128 - p.j * NS
                if o >= 0:  # diagonal block: causal mask
                    nc.vector.tensor_mul(
                        ptw[:, lg * NS:(lg + 1) * NS],
                        ptw[:, lg * NS:(lg + 1) * NS],
                        msk[:, (NS - 128) - o:(2 * NS - 128) - o])
            if gi == 0:
                p.denw = den_pool.tile([128, 3 * NS], F16, tag="den",
                                       name=f"den_{p.j}_{p.h}")
                nc.vector.tensor_copy(p.denw[:, 0:G * NS], ptw[:])
                p.dw = G
            else:
                ga = min(G, p.dw)
                nc.vector.tensor_add(p.denw[:, 0:ga * NS],
                                     p.denw[:, 0:ga * NS],
                                     ptw[:, 0:ga * NS])
                if G > p.dw:
                    nc.vector.tensor_copy(p.denw[:, p.dw * NS:G * NS],
                                          ptw[:, ga * NS:G * NS])
                    p.dw = G

        def av_tail(p, gi, si0, G, ptw):
            """AV accumulation MMs for one group (PE, lag 2)."""
            if gi == 0:
                p.at_ps = atp_pool.tile([128, NS], F32, tag="atp",
                                        name=f"at_{p.j}_{p.h}")
            for lg in range(G):
                si = si0 + lg
                nc.tensor.matmul(
                    p.at_ps[:], vnat[:, si * 128:(si + 1) * 128],
                    ptw[:, lg * NS:(lg + 1) * NS],
                    start=(si == 0), stop=(si == p.nsk - 1))

        def fin_a(p):
            """den fold + partition-sum + reciprocal + broadcast."""
            nslots = p.dw
            if nslots >= 2:
                nc.vector.tensor_add(p.denw[:, 0:NS], p.denw[:, 0:NS],
                                     p.denw[:, NS:2 * NS])
            if nslots >= 3:
                nc.vector.tensor_add(p.denw[:, 0:NS], p.denw[:, 0:NS],
                                     p.denw[:, 2 * NS:3 * NS])
            dps = dps_pool.tile([1, NS], F32, tag="dps",
                                name=f"dps_{p.j}_{p.h}")
            nc.tensor.matmul(dps[:], ones[:, 0:1], p.denw[:, 0:NS],
                             start=True, stop=True)
            rc = rc_pool.tile([1, NS], F32, tag="rc")
            nc.vector.reciprocal_approx_fast(out=rc[:], in_=dps[:])
            rbc = rbc_pool.tile([128, NS], F32, tag="rbc")
            nc.gpsimd.partition_broadcast(rbc[:], rc[:])
            p.rbc = rbc

        def fin_b(p):
            """normalize + export strips + (maybe) trigger AllToAll."""
            asb = attn_pool.tile([128, NS], BF16, tag="attn")
            nc.vector.tensor_mul(asb[:], p.at_ps[:], p.rbc[:])
            c0 = p.j * NS
            dd0 = (c0 // SW) % N_CORES
            m = c0 // (N_CORES * SW)
            dst = a2a_in[m].ap().rearrange("(d p) w -> p d w", p=MQ)
            nc.sync.dma_start(
                dst[p.h * hd:(p.h + 1) * hd, dd0:dd0 + NSW, :],
                asb[:].rearrange("p (d w) -> p d w", d=NSW))
            if p.j % (NC // NM) == (NC // NM) - 1 and p.h == HQL - 1:
                nc.gpsimd.collective_compute(
                    "AllToAll", mybir.AluOpType.bypass,
                    ins=[a2a_in[m].ap().opt()],
                    outs=[a2a_out[m].ap().opt()],
                    replica_groups=groups)

        # flat emission pipeline: score+exp(g) | tail(g-1) | deferred fins
        items = []
        g_global = 0
        for j in range(NC):
            for h in range(HQL):
                p = Pass(j, h, g_global)
                g_global = p.g_end
                si0 = 0
                for gi, (gidx, G) in enumerate(p.gs):
                    items.append((p, gidx, gi, si0, G))
                    si0 += G
        q = []      # emitted score groups: den at lag 1, AV at lag 2
        fins = []   # list of [pass, next_stage] with stage in ("a","b")

        def advance_fins():
            adv = 0
            while fins and (adv == 0 or (len(fins) > 1 and adv < 3)):
                fp, stage = fins[0]
                if stage == "a":
                    fin_a(fp)
                    fins[0][1] = "b"
                else:
                    fin_b(fp)
                    fins.pop(0)
                adv += 1

        def emit_av(entry):
            tp = entry[0]
            av_tail(*entry)
            if entry[1] == len(tp.gs) - 1:
                fins.append([tp, "a"])

        for it in items:
            p, gidx, gi, si0, G = it
            ptw = score_group(p, gidx, gi, si0, G)
            if len(q) >= 2:
                emit_av(q[-2])
            if len(q) >= 1:
                den_tail(*q[-1])
            q.append((p, gi, si0, G, ptw))
            if len(q) > 2:
                q.pop(0)
            advance_fins()
        if len(q) >= 2:
            emit_av(q[-2])
        den_tail(*q[-1])
        emit_av(q[-1])
        for fp, stage in fins:
            if stage == "a":
                fin_a(fp)
            fin_b(fp)

        # ---- phase 3: output projection ----
        NO = D // NS
        for m in range(NM):
            for beta in range(NB):
                aot = aot_pool.tile([128, NH * SW], BF16, tag="aot",
                                    name=f"aot_{m}_{beta}")
                src = a2a_out[m].ap()[beta * TP * MQ:(beta + 1) * TP * MQ,
                                      :].rearrange("(t p) w -> p t w", p=hd)
                nc.sync.dma_start(
                    aot[:].rearrange("p (t w) -> p t w", t=NH), src)
                for n in range(NO):
                    pso = dps_pool.tile([128, NS], F32, tag="dps",
                                        name=f"pso_{m}_{beta}_{n}")
                    for ht in range(NH):
                        nc.tensor.matmul(
                            pso[:], aot[:, ht * SW:(ht + 1) * SW],
                            wo_tiles[ht][:, n * NS:(n + 1) * NS],
                            start=(ht == 0), stop=(ht == NH - 1))
                    ob = osb_pool.tile([128, NS], F32, tag="osb")
                    nc.vector.tensor_copy(ob[:], pso[:])
                    r0 = beta * OW + m * SW
                    nc.sync.dma_start(
                        out_e[r0:r0 + SW, n * NS:(n + 1) * NS],
                        ob[0:SW, :])

    nc.compile()
    return nc


def host_prepare(x, wq, wk, wv, wo, S, D, HQL, NS):
    """Layout-only host prep: slice/transpose/cast + rope tables + mask."""
    hd = HD
    MQ = HQL * hd
    bf = ml_dtypes.bfloat16

    perm = np.concatenate([np.arange(0, hd, 2), np.arange(1, hd, 2)])

    def permute_heads(w):
        nh = w.shape[0] // hd
        w = w.reshape(nh, hd, -1)[:, perm, :]
        return w.reshape(nh * hd, -1)

    wq_p = permute_heads(wq)
    wk_p = permute_heads(wk)

    inv_freq = 1.0 / (ROPE_THETA ** (np.arange(0, hd, 2, dtype=np.float64)
                                     / hd))
    ang = np.arange(S, dtype=np.float64)[None, :] * inv_freq[:, None]
    cc = np.cos(ang)
    ss = np.sin(ang)
    # rope tables over full 128 partitions: rows 0:64 = even dims,
    # rows 64:128 = odd dims.  out = t*ccx + swap(t)*ssx.
    ccx = np.concatenate([cc, cc], axis=0).astype(bf)
    ssx = np.concatenate([-ss, ss], axis=0).astype(bf)

    pswap = np.zeros((128, 128), dtype=bf)
    pswap[np.arange(64), np.arange(64, 128)] = 1.0
    pswap[np.arange(64, 128), np.arange(64)] = 1.0

    p = np.arange(128)[:, None]
    c = np.arange(NS + 384)[None, :]
    mski = (p <= c - (NS - 128)).astype(np.float16)

    woT = np.ascontiguousarray(wo.T).astype(bf)

    in_maps = []
    for core in range(N_CORES):
        b = core // TP
        r = core % TP
        qsl = slice(r * MQ, (r + 1) * MQ)
        ksl = slice(r * hd, (r + 1) * hd)
        in_maps.append({
            "xT": np.ascontiguousarray(x[b].T).astype(bf),
            "wqT": np.ascontiguousarray(wq_p[qsl].T).astype(bf),
            "wkT": np.ascontiguousarray(wk_p[ksl].T).astype(bf),
            "wvT": np.ascontiguousarray(wv[ksl].T).astype(bf),
            "woT": woT,
            "ccx": ccx, "ssx": ssx, "pswap": pswap, "mask": mski,
        })
    return in_maps


_NC_CACHE = {}


def get_graph(S=2048, D=2048, HQL=4, NS=512):
    key = (S, D, HQL, NS)
    if key not in _NC_CACHE:
        _NC_CACHE[key] = build_graph(S, D, HQL, NS)
    return _NC_CACHE[key]


def unshard_out(results, B, S, D):
    """results[core]["out"] is [NB*OW, D] with rows (beta, strip m, 128)."""
    out = np.empty((B, S, D), dtype=np.float32)
    OW = S // N_CORES
    NM = max(1, S // (N_CORES * 128))
    SW = OW // NM
    for core in range(N_CORES):
        r = results[core]["out"]
        for beta in range(B):
            for m in range(NM):
                c0 = core * SW + m * N_CORES * SW
                out[beta, c0:c0 + SW, :] = \
                    r[beta * OW + m * SW:beta * OW + (m + 1) * SW, :]
    return out


def kernel(x, wq, wk, wv, wo, trace=False):
    B, S, D = x.shape
    HQL = (wq.shape[0] // HD) // TP
    NS = 512
    nc = get_graph(S, D, HQL, NS)
    in_maps = host_prepare(x, wq, wk, wv, wo, S, D, HQL, NS)
    res = run_bass_kernel_spmd(nc, in_maps, core_ids=list(range(N_CORES)),
                               trace=trace)
    out = unshard_out(res.results, B, S, D)
    if trace:
        kernel.last_exec_time_ns = res.exec_time_ns
        kernel.last_results = res
    return out


# revision 24
# speedup vs baseline: 1.0518x; 1.0518x over previous
"""Trainium2 Bass kernel for GQA causal attention (B=2, S=2048, D=2048,
16 q-heads / 4 kv-heads, head_dim=128, interleaved RoPE).

Sharding: DP=2 over batch x TP=4 over head groups (8 cores).
Core c: batch b=c//4, rank r=c%4 -> q-heads [4r,4r+4), kv-head r.
Each core computes its heads' attention output (transposed layout [e,s]),
two column-strip AllToAlls reshard heads->sequence (overlapped with the
tail of attention), and each core runs the full output projection for its
512 strided sequence rows. Host-side work is layout only: slicing,
transposing, bf16 casting.

v2 changes vs baseline (trace-driven):
 - RoPE: partition-half swap via one PE permutation matmul instead of
   SBUF->SBUF DMAs; 3 full-height bf16 DVE ops (2x mode) instead of six
   half-height fp32 ops (1x).
 - softmax denominator: fp16 accumulate on wide group tiles (2x DVE),
   reciprocal via reciprocal_approx_fast (was 4us/call DVE reciprocal).
 - scores: 3-bank / 2-bank wide PSUM group tiles, ONE exp ACTIVATE per
   group (amortizes the 352-cycle ACT overhead).
 - exp ACT table preloaded at graph start.
 - a2a export/import as single rearranged-AP DMAs instead of 4-16 small
   DMAs (Sync engine issue cost).
"""

import math
import sys

sys.path.insert(0, "/opt/trn_rl_repo")

from contextlib import ExitStack

import ml_dtypes
import numpy as np

import concourse.bass as bass
import concourse.mybir as mybir
import concourse.tile as tile
from concourse import bacc
from concourse.bass_utils import run_bass_kernel_spmd
from concourse.masks import make_identity

BF16 = mybir.dt.bfloat16
F16 = mybir.dt.float16
F32 = mybir.dt.float32

N_HEADS = 16
N_KV_HEADS = 4
HD = 128
ROPE_THETA = 10000.0
TP = 4
N_CORES = 8


def build_graph(S=2048, D=2048, HQL=4, NS=512):
    """Per-core SPMD graph. HQL = local q heads; local kv heads = 1.

    Output ownership is strided by 128-col strips: core c owns sequence
    cols {c*128 + m*1024} of both batches; strip set m is exchanged by
    AllToAll #m as soon as the first half of attention chunks finish.
    """
    hd = HD
    ND = D // 128          # d-tiles (projection contraction tiles)
    NC = S // NS           # s-chunks
    NK = S // 128          # sk-tiles
    MQ = HQL * hd          # local q width
    DIAG = NS // 128       # sk-tiles per chunk needing a causal mask
    NB = N_CORES // TP     # batches
    OW = S // N_CORES      # out cols per core per batch
    NM = max(1, S // (N_CORES * 128))   # strips (AllToAll count)
    SW = OW // NM          # strip width (=128 at full size)
    NSW = NS // SW         # strips exported per pass
    scale = 1.0 / math.sqrt(hd)
    NH = TP * HQL          # global head count

    nc = bacc.Bacc("TRN2", target_bir_lowering=False, debug=False,
                   num_devices=N_CORES)

    xT_e = nc.dram_tensor("xT", [D, S], BF16, kind="ExternalInput").ap()
    wqT_e = nc.dram_tensor("wqT", [D, MQ], BF16, kind="ExternalInput").ap()
    wkT_e = nc.dram_tensor("wkT", [D, hd], BF16, kind="ExternalInput").ap()
    wvT_e = nc.dram_tensor("wvT", [D, hd], BF16, kind="ExternalInput").ap()
    woT_e = nc.dram_tensor("woT", [NH * hd, D], BF16,
                           kind="ExternalInput").ap()
    ccx_e = nc.dram_tensor("ccx", [128, S], BF16, kind="ExternalInput").ap()
    ssx_e = nc.dram_tensor("ssx", [128, S], BF16, kind="ExternalInput").ap()
    psw_e = nc.dram_tensor("pswap", [128, 128], BF16,
                           kind="ExternalInput").ap()
    mask_e = nc.dram_tensor("mask", [128, NS + 384], F16,
                            kind="ExternalInput").ap()
    out_e = nc.dram_tensor("out", [NB * OW, D], F32,
                           kind="ExternalOutput").ap()

    a2a_in = [nc.dram_tensor(f"a2a_in{m}", [N_CORES * MQ, SW], BF16)
              for m in range(NM)]
    a2a_out = [nc.dram_tensor(f"a2a_out{m}", [N_CORES * MQ, SW], BF16)
               for m in range(NM)]
    groups = [list(range(N_CORES))]

    with tile.TileContext(nc) as tc, ExitStack() as ctx:
        ep = ctx.enter_context
        const_pool = ep(tc.tile_pool(name="const", bufs=1))
        rt_pool = ep(tc.tile_pool(name="rt", bufs=HQL + 1))
        vst_pool = ep(tc.tile_pool(name="vst", bufs=1))
        ptw_pool = ep(tc.tile_pool(name="ptw", bufs=3))
        den_pool = ep(tc.tile_pool(name="den", bufs=3))
        rc_pool = ep(tc.tile_pool(name="rc", bufs=3))
        rbc_pool = ep(tc.tile_pool(name="rbc", bufs=3))
        attn_pool = ep(tc.tile_pool(name="attn", bufs=4))
        osb_pool = ep(tc.tile_pool(name="osb", bufs=3))
        # PSUM: 8 banks total, statically partitioned:
        #   scw3 1x3 banks + scw2 1x2 + atp 2x1 + dps 1x1 = 8
        scw3_pool = ep(tc.tile_pool(name="scw3", bufs=1, space="PSUM"))
        scw2_pool = ep(tc.tile_pool(name="scw2", bufs=1, space="PSUM"))
        atp_pool = ep(tc.tile_pool(name="atp", bufs=2, space="PSUM"))
        dps_pool = ep(tc.tile_pool(name="dps", bufs=1, space="PSUM"))

        # ---- constants ----
        ident = const_pool.tile([128, 128], BF16, tag="ident")
        make_identity(nc, ident[:])
        ones = const_pool.tile([128, 1], F16, tag="ones")
        nc.gpsimd.memset(ones[:], 1.0)
        warm = const_pool.tile([1, 8], F32, tag="warm")
        nc.gpsimd.memset(warm[:], 0.0)
        # preload the exp ACT table set before attention needs it
        nc.scalar.activation(warm[:], warm[:],
                             mybir.ActivationFunctionType.Exp)
        ccx = const_pool.tile([128, S], BF16, tag="ccx")
        ssx = const_pool.tile([128, S], BF16, tag="ssx")
        pswap = const_pool.tile([128, 128], BF16, tag="pswap")
        msk = const_pool.tile([128, NS + 384], F16, tag="msk")

        rts = []
        vst = vst_pool.tile([128, S], BF16, tag="vst")   # vT staging
        vnat = vst_pool.tile([128, S], BF16, tag="vnat")  # v [sk, e] blocks

        # ---- phase 1: projections + rope (xt pools close after) ----
        with tc.tile_pool(name="xt", bufs=ND) as xt_pool, \
             tc.tile_pool(name="wq", bufs=ND) as wq_pool, \
             tc.tile_pool(name="wkv", bufs=2 * ND) as wkv_pool, \
             tc.tile_pool(name="stg", bufs=2) as stage_pool, \
             tc.tile_pool(name="tmp", bufs=4) as tmp_pool:
            xts, wqs, wks, wvs = [], [], [], []
            # xt+wq interleaved first (q heads project first and are
            # DMA-gated); v/k weights after
            # xt tiles are loaded in chunk-column pieces, chunk-major,
            # so the first projection group (chunk 0) is runnable after
            # ~2MB instead of the full 8MB of xT
            for d in range(ND):
                xts.append(xt_pool.tile([128, S], BF16, tag="xt",
                                        name=f"xt{d}"))
                wqs.append(wq_pool.tile([128, MQ], BF16, tag="wq",
                                        name=f"wq{d}"))
            for sp in range(NC):
                for d in range(ND):
                    nc.sync.dma_start(
                        xts[d][:, sp * NS:(sp + 1) * NS],
                        xT_e[d * 128:(d + 1) * 128,
                             sp * NS:(sp + 1) * NS])
                    if sp == 0:
                        nc.sync.dma_start(
                            wqs[d][:], wqT_e[d * 128:(d + 1) * 128, :])
                    if sp == 0 and d == 0:
                        nc.sync.dma_start(ccx[:], ccx_e[:])
                        nc.sync.dma_start(ssx[:], ssx_e[:])
                        nc.sync.dma_start(pswap[:], psw_e[:])
                        nc.sync.dma_start(msk[:], mask_e[:])
            for d in range(ND):
                wv = wkv_pool.tile([128, hd], BF16, tag="wkv",
                                   name=f"wv{d}")
                nc.sync.dma_start(wv[:], wvT_e[d * 128:(d + 1) * 128, :])
                wvs.append(wv)
            for d in range(ND):
                wk = wkv_pool.tile([128, hd], BF16, tag="wkv",
                                   name=f"wk{d}")
                nc.sync.dma_start(wk[:], wkT_e[d * 128:(d + 1) * 128, :])
                wks.append(wk)

            # Rope is software-pipelined one m-tile behind the
            # projection matmuls: the swap-MM of m-tile k is emitted
            # after the proj group of m-tile k+1 so it never stalls the
            # PE waiting for the ACT psum->stg copy.
            rope_pend = []   # stack of (stg, ps2 emitter, rt_tile, ssl)

            def rope_tail():
                if not rope_pend:
                    return
                stg, ps2, rt_tile, ssl = rope_pend.pop(0)
                t1 = tmp_pool.tile([128, NS], BF16, tag="tmp")
                nc.vector.tensor_mul(t1[:], stg[:], ccx[:, ssl])
                t2 = tmp_pool.tile([128, NS], BF16, tag="tmp")
                nc.vector.tensor_mul(t2[:], ps2[:], ssx[:, ssl])
                nc.vector.tensor_add(rt_tile[:, ssl], t1[:], t2[:])

            def rope_swap_mm():
                if not rope_pend:
                    return
                stg, _, rt_tile, ssl = rope_pend[0]
                ps2 = scw2_pool.tile([128, NS], F32, tag="scw2",
                                     name="ps_swap")
                nc.tensor.matmul(ps2[:], pswap[:], stg[:],
                                 start=True, stop=True)
                rope_pend[0] = (stg, ps2, rt_tile, ssl)

            def proj_mtile(lhs_tiles, mslice, is_v, rt_tile):
                for s in range(NC):
                    ssl = slice(s * NS, (s + 1) * NS)
                    ps = atp_pool.tile([128, NS], F32, tag="atp",
                                       name="psp")
                    for d in range(ND):
                        nc.tensor.matmul(
                            ps[:], lhs_tiles[d][:, mslice],
                            xts[d][:, ssl],
                            start=(d == 0), stop=(d == ND - 1))
                    # emit the PREVIOUS m-tile's swap-MM now (its stg
                    # copy has had a full m-tile of time to finish)
                    rope_swap_mm()
                    rope_tail()
                    if is_v:
                        nc.scalar.copy(vst[:, ssl], ps[:])
                    else:
                        stg = stage_pool.tile([128, NS], BF16, tag="stg")
                        nc.scalar.copy(stg[:], ps[:])
                        rope_pend.append((stg, None, rt_tile, ssl))

            # q heads first, then v, then k LAST: attention chunk j only
            # needs k-rope of chunk j, so scores start right after k's
            # first chunk instead of after the whole projection phase
            for h in range(HQL):
                rt = rt_pool.tile([128, S], BF16, tag="rt", name=f"rtq{h}")
                proj_mtile(wqs, slice(h * hd, (h + 1) * hd), False, rt)
                rts.append(rt)
            proj_mtile(wvs, slice(0, hd), True, None)
            # v transpose: vst [e, s] -> vnat [sk, e] blocks
            for st in range(NK):
                tpp = dps_pool.tile([128, 128], BF16, tag="dps",
                                    name="pst")
                if st == 0:
                    rope_swap_mm()
                    rope_tail()
                nc.tensor.transpose(
                    tpp[:], vst[:, st * 128:(st + 1) * 128], ident[:])
                nc.scalar.copy(vnat[:, st * 128:(st + 1) * 128], tpp[:])
            krt = rt_pool.tile([128, S], BF16, tag="rt", name="rtk")
            proj_mtile(wks, slice(0, hd), False, krt)
            rope_swap_mm()
            rope_tail()

        # ---- woT preload (streams during attention; reuses xt space) ----
        wo_pool = ep(tc.tile_pool(name="wo", bufs=NH))
        aot_pool = ep(tc.tile_pool(name="aot", bufs=2))
        wo_tiles = []
        for ht in range(NH):
            w = wo_pool.tile([128, D], BF16, tag="wo", name=f"wo{ht}")
            nc.sync.dma_start(w[:], woT_e[ht * 128:(ht + 1) * 128, :])
            wo_tiles.append(w)

        # ---- phase 2: attention ----
        # One head per pass; sk-tiles processed in wide groups that
        # alternate between a 3-bank and a 2-bank PSUM tile, with ONE
        # exp ACTIVATE per group. Score groups are emitted one step
        # ahead of the exp/mask/den/AV tail so the PE never waits on
        # ACT; a pass's finalize (den fold+reduce, recip, broadcast)
        # and export (normalize + DMA + a2a) are deferred into the next
        # pass's first two groups.
        class Pass:
            def __init__(self, j, h, g0):
                self.j = j
                self.h = h
                self.nsk = (j + 1) * DIAG
                # group sizes follow the GLOBAL 3/2 pool alternation so
                # consecutive passes never collide on the same PSUM slot
                self.gs = []
                rem = self.nsk
                g = g0
                while rem > 0:
                    take = min(3 if g % 2 == 0 else 2, rem)
                    self.gs.append((g, take))
                    rem -= take
                    g += 1
                self.g_end = g
                self.dw = 0      # initialized width (slots) of denw
                self.denw = None
                self.at_ps = None
                self.rbc = None

        def score_group(p, gidx, gi, si0, G):
            """score MMs + exp for one group; ACT starts ASAP."""
            pool = scw3_pool if gidx % 2 == 0 else scw2_pool
            tag = "scw3" if gidx % 2 == 0 else "scw2"
            scw = pool.tile([128, G * NS], F32, tag=tag,
                            name=f"sc_{p.j}_{p.h}_{gi}")
            for lg in range(G):
                si = si0 + lg
                nc.tensor.matmul(
                    scw[:, lg * NS:(lg + 1) * NS],
                    krt[:, si * 128:(si + 1) * 128],
                    rts[p.h][:, p.j * NS:(p.j + 1) * NS],
                    start=True, stop=True)
            ptw = ptw_pool.tile([128, G * NS], F16, tag="ptw")
            nc.scalar.activation(ptw[:], scw[:],
                                 mybir.ActivationFunctionType.Exp,
                                 scale=scale)
            return ptw

        def tail_group(p, gi, si0, G, ptw):
            """mask + den-accumulate + AV for one group (lag 1)."""
            for lg in range(G):
                si = si0 + lg
                o = si * 128 - p.j * NS
                if o >= 0:  # diagonal block: causal mask
                    nc.vector.tensor_mul(
                        ptw[:, lg * NS:(lg + 1) * NS],
                        ptw[:, lg * NS:(lg + 1) * NS],
                        msk[:, (NS - 128) - o:(2 * NS - 128) - o])
            if gi == 0:
                p.denw = den_pool.tile([128, 3 * NS], F16, tag="den",
                                       name=f"den_{p.j}_{p.h}")
                nc.vector.tensor_copy(p.denw[:, 0:G * NS], ptw[:])
                p.dw = G
            else:
                ga = min(G, p.dw)
                nc.vector.tensor_add(p.denw[:, 0:ga * NS],
                                     p.denw[:, 0:ga * NS],
                                     ptw[:, 0:ga * NS])
                if G > p.dw:
                    nc.vector.tensor_copy(p.denw[:, p.dw * NS:G * NS],
                                          ptw[:, ga * NS:G * NS])
                    p.dw = G
            if gi == 0:
                p.at_ps = atp_pool.tile([128, NS], F32, tag="atp",
                                        name=f"at_{p.j}_{p.h}")
            for lg in range(G):
                si = si0 + lg
                nc.tensor.matmul(
                    p.at_ps[:], vnat[:, si * 128:(si + 1) * 128],
                    ptw[:, lg * NS:(lg + 1) * NS],
                    start=(si == 0), stop=(si == p.nsk - 1))

        def fin_a(p):
            """den fold + partition-sum + reciprocal + broadcast."""
            nslots = p.dw
            if nslots >= 2:
                nc.vector.tensor_add(p.denw[:, 0:NS], p.denw[:, 0:NS],
                                     p.denw[:, NS:2 * NS])
            if nslots >= 3:
                nc.vector.tensor_add(p.denw[:, 0:NS], p.denw[:, 0:NS],
                                     p.denw[:, 2 * NS:3 * NS])
            dps = dps_pool.tile([1, NS], F32, tag="dps",
                                name=f"dps_{p.j}_{p.h}")
            nc.tensor.matmul(dps[:], ones[:, 0:1], p.denw[:, 0:NS],
                             start=True, stop=True)
            rc = rc_pool.tile([1, NS], F32, tag="rc")
            nc.vector.reciprocal_approx_fast(out=rc[:], in_=dps[:])
            rbc = rbc_pool.tile([128, NS], F32, tag="rbc")
            nc.gpsimd.partition_broadcast(rbc[:], rc[:])
            p.rbc = rbc

        def fin_b(p):
            """normalize + export strips + (maybe) trigger AllToAll."""
            asb = attn_pool.tile([128, NS], BF16, tag="attn")
            nc.vector.tensor_mul(asb[:], p.at_ps[:], p.rbc[:])
            c0 = p.j * NS
            dd0 = (c0 // SW) % N_CORES
            m = c0 // (N_CORES * SW)
            dst = a2a_in[m].ap().rearrange("(d p) w -> p d w", p=MQ)
            nc.sync.dma_start(
                dst[p.h * hd:(p.h + 1) * hd, dd0:dd0 + NSW, :],
                asb[:].rearrange("p (d w) -> p d w", d=NSW))
            if p.j % (NC // NM) == (NC // NM) - 1 and p.h == HQL - 1:
                nc.gpsimd.collective_compute(
                    "AllToAll", mybir.AluOpType.bypass,
                    ins=[a2a_in[m].ap().opt()],
                    outs=[a2a_out[m].ap().opt()],
                    replica_groups=groups)

        # flat emission pipeline: score+exp(g) | tail(g-1) | deferred fins
        items = []
        g_global = 0
        for j in range(NC):
            for h in range(HQL):
                p = Pass(j, h, g_global)
                g_global = p.g_end
                si0 = 0
                for gi, (gidx, G) in enumerate(p.gs):
                    items.append((p, gidx, gi, si0, G))
                    si0 += G
        pend_tail = None
        fins = []   # list of [pass, next_stage] with stage in ("a","b")
        for it in items:
            p, gidx, gi, si0, G = it
            ptw = score_group(p, gidx, gi, si0, G)
            if pend_tail is not None:
                tp = pend_tail[0]
                tail_group(*pend_tail)
                if pend_tail[1] == len(tp.gs) - 1:
                    fins.append([tp, "a"])
            pend_tail = (p, gi, si0, G, ptw)
            # advance deferred finalize stages: one per group in steady
            # state, more if a backlog builds
            adv = 0
            while fins and (adv == 0 or (len(fins) > 1 and adv < 3)):
                fp, stage = fins[0]
                if stage == "a":
                    fin_a(fp)
                    fins[0][1] = "b"
                else:
                    fin_b(fp)
                    fins.pop(0)
                adv += 1
        tp = pend_tail[0]
        tail_group(*pend_tail)
        fins.append([tp, "a"])
        for fp, stage in fins:
            if stage == "a":
                fin_a(fp)
            fin_b(fp)

        # ---- phase 3: output projection ----
        NO = D // NS
        for m in range(NM):
            for beta in range(NB):
                aot = aot_pool.tile([128, NH * SW], BF16, tag="aot",
                                    name=f"aot_{m}_{beta}")
                src = a2a_out[m].ap()[beta * TP * MQ:(beta + 1) * TP * MQ,
                                      :].rearrange("(t p) w -> p t w", p=hd)
                nc.sync.dma_start(
                    aot[:].rearrange("p (t w) -> p t w", t=NH), src)
                for n in range(NO):
                    pso = atp_pool.tile([128, NS], F32, tag="atp",
                                        name=f"pso_{m}_{beta}_{n}")
                    for ht in range(NH):
                        nc.tensor.matmul(
                            pso[:], aot[:, ht * SW:(ht + 1) * SW],
                            wo_tiles[ht][:, n * NS:(n + 1) * NS],
                            start=(ht == 0), stop=(ht == NH - 1))
                    ob = osb_pool.tile([128, NS], F32, tag="osb")
                    nc.vector.tensor_copy(ob[:], pso[:])
                    r0 = beta * OW + m * SW
                    nc.sync.dma_start(
                        out_e[r0:r0 + SW, n * NS:(n + 1) * NS],
                        ob[0:SW, :])

    nc.compile()
    return nc


def host_prepare(x, wq, wk, wv, wo, S, D, HQL, NS):
    """Layout-only host prep: slice/transpose/cast + rope tables + mask."""
    hd = HD
    MQ = HQL * hd
    bf = ml_dtypes.bfloat16

    perm = np.concatenate([np.arange(0, hd, 2), np.arange(1, hd, 2)])

    def permute_heads(w):
        nh = w.shape[0] // hd
        w = w.reshape(nh, hd, -1)[:, perm, :]
        return w.reshape(nh * hd, -1)

    wq_p = permute_heads(wq)
    wk_p = permute_heads(wk)

    inv_freq = 1.0 / (ROPE_THETA ** (np.arange(0, hd, 2, dtype=np.float64)
                                     / hd))
    ang = np.arange(S, dtype=np.float64)[None, :] * inv_freq[:, None]
    cc = np.cos(ang)
    ss = np.sin(ang)
    # rope tables over full 128 partitions: rows 0:64 = even dims,
    # rows 64:128 = odd dims.  out = t*ccx + swap(t)*ssx.
    ccx = np.concatenate([cc, cc], axis=0).astype(bf)
    ssx = np.concatenate([-ss, ss], axis=0).astype(bf)

    pswap = np.zeros((128, 128), dtype=bf)
    pswap[np.arange(64), np.arange(64, 128)] = 1.0
    pswap[np.arange(64, 128), np.arange(64)] = 1.0

    p = np.arange(128)[:, None]
    c = np.arange(NS + 384)[None, :]
    mski = (p <= c - (NS - 128)).astype(np.float16)

    woT = np.ascontiguousarray(wo.T).astype(bf)

    in_maps = []
    for core in range(N_CORES):
        b = core // TP
        r = core % TP
        qsl = slice(r * MQ, (r + 1) * MQ)
        ksl = slice(r * hd, (r + 1) * hd)
        in_maps.append({
            "xT": np.ascontiguousarray(x[b].T).astype(bf),
            "wqT": np.ascontiguousarray(wq_p[qsl].T).astype(bf),
            "wkT": np.ascontiguousarray(wk_p[ksl].T).astype(bf),
            "wvT": np.ascontiguousarray(wv[ksl].T).astype(bf),
            "woT": woT,
            "ccx": ccx, "ssx": ssx, "pswap": pswap, "mask": mski,
        })
    return in_maps


_NC_CACHE = {}


def get_graph(S=2048, D=2048, HQL=4, NS=512):
    key = (S, D, HQL, NS)
    if key not in _NC_CACHE:
        _NC_CACHE[key] = build_graph(S, D, HQL, NS)
    return _NC_CACHE[key]


def unshard_out(results, B, S, D):
    """results[core]["out"] is [NB*OW, D] with rows (beta, strip m, 128)."""
    out = np.empty((B, S, D), dtype=np.float32)
    OW = S // N_CORES
    NM = max(1, S // (N_CORES * 128))
    SW = OW // NM
    for core in range(N_CORES):
        r = results[core]["out"]
        for beta in range(B):
            for m in range(NM):
                c0 = core * SW + m * N_CORES * SW
                out[beta, c0:c0 + SW, :] = \
                    r[beta * OW + m * SW:beta * OW + (m + 1) * SW, :]
    return out


def kernel(x, wq, wk, wv, wo, trace=False):
    B, S, D = x.shape
    HQL = (wq.shape[0] // HD) // TP
    NS = 512
    nc = get_graph(S, D, HQL, NS)
    in_maps = host_prepare(x, wq, wk, wv, wo, S, D, HQL, NS)
    res = run_bass_kernel_spmd(nc, in_maps, core_ids=list(range(N_CORES)),
                               trace=trace)
    out = unshard_out(res.results, B, S, D)
    if trace:
        kernel.last_exec_time_ns = res.exec_time_ns
        kernel.last_results = res
    return out
